# revision 22
# baseline (speedup 1.0000x reference)
"""Single-head causal attention (B=8, S=2048, E=2048, D=128) on 8 trn2 cores.

Sharding: data-parallel over batch — one batch element per NeuronCore.

Host marshaling per core: xT = x[b].T cast to fp16; the q/k/v weights are
transposed, cast, and packed into single tensors in the exact SBUF tile
layout (contiguous line-rate DMAs), the three biases into ONE [128, 3]
tensor.

Projection precision modes (PROJ_DTYPE):
  - "fp16": all three projections fp16 (1 col/cycle PE rate)
  - "fp8":  all three projections fp8e4m3 DoubleRow (2x PE rate, v-path
            quantization error ~3e-2 — fails tight tolerances)
  - "mixed" (default): q,k projections fp8 DoubleRow, v projection fp16.
            The fp8 error only perturbs softmax logits, which are scaled
            by 1/sqrt(2048) — output error stays ~4e-3 while 2/3 of the
            projection FLOPs run at 2x rate. With CAST_ON_DEVICE (default)
            the fp8 copy of x is produced by the DVE from the fp16 stream
            (hidden under the DMA), so HBM traffic stays at the fp16
            baseline; otherwise xT ships twice (fp8 + fp16).

Per-core dataflow (f32 PSUM accumulation):
  - projections produce qT/kT/vT in [D, S] layout; bias added during the
    VectorE PSUM->SBUF evacuation (per-partition scalar add)
  - vT is re-transposed on the PE into natural [S, D] blocks, augmented
    with a ones column (col 128): the AV matmul then yields the softmax
    denominator for free as output column 128
  - scoresT[k, q] per k-block j: single matmul (K=D=128), exact causal
    trim of the q range; diagonal 128-block masked by adding -1e30;
    ScalarE computes exp(scale*s) straight out of PSUM into fp16 probsT.
    The score stream is paced by these ScalarE evacuations (~3x the PE
    cost per piece), so the independent v-projection chunks are
    interleaved between score pieces to keep the in-order PE queue fed
  - AV per q-block i accumulates probsT_j.T @ v_aug_j over j<=i in PSUM;
    VectorE takes 1/denominator and applies it during the final
    evacuation; one batched DMA per 512-row chunk stores fp16 output
    (host upcasts to f32)

Loop-timing structure: the For_i body is unrolled 4x — each For_i
iteration carries an all-engine barrier in its reset block (~6us of
drain + pipeline/DMA-prefetch refill), so fewer, fatter iterations
amortize it. qT/kT/vT/v and x tiles are multi-buffered so consecutive
bodies overlap (next body's DMA/casts/projections run under the current
body's scores/AV tail).
"""

import math
import os

import numpy as np

B = 8
S = 2048
E = 2048
D = 128
P = 128
NE = E // P  # 16 contraction chunks
NS = S // P  # 16 sequence blocks
ST = 512  # s-tile width for projections / score chunks
NST = S // ST  # 4
VW = D + 1  # logical v block width incl. ones column
VSTRIDE = D + 1  # physical stride of v blocks in SBUF
SCALE = 1.0 / math.sqrt(S)
NEG = -1.0e30

_PROGRAMS = {}

# which phases to emit (for microbenchmarking): subset of
# {"proj", "vtrans", "scores", "av", "store"}
PHASES = frozenset(
    p
    for p in os.environ.get("K_PHASES", "proj,vtrans,scores,av,store").split(",")
    if p
)

# per-projection matmul precision; see module docstring
PROJ_DTYPE = os.environ.get("K_PROJ_DTYPE", "mixed")
W_SCALE = 256.0  # host pre-scale of W before fp8 quantization (2**8: exact)
# produce the fp8 x copy on-device (Pool engine) instead of shipping it
CAST_ON_DEVICE = os.environ.get("K_CAST", "1") == "1"
# comma-separated rotation of engines hosting the cast ops
CAST_ENG = os.environ.get("K_CAST_ENG", "vector")

# tunables: PSUM bank split (proj, sc, out; vt fixed at 1) and probs pool depth
PS_CFG = tuple(int(v) for v in os.environ.get("K_PS_CFG", "3,2,2").split(","))
# v natural-block transpose: "pe" (tensor-engine transpose + DVE copy) or
# "dma" (SBUF->SBUF DMA transpose, frees PE/DVE/PSUM)
VTRANS = os.environ.get("K_VTRANS", "pe")
VTRANS_DMA = os.environ.get("K_VTRANS_DMA", "sync")
PPOOL = int(os.environ.get("K_PPOOL", "24"))
OUT_DMA = os.environ.get("K_OUT_DMA", "gpsimd")  # engine hosting output stores
OUT_DTYPE = os.environ.get("K_OUT_DTYPE", "fp16")  # f32|fp16 store dtype
X_DMA = os.environ.get("K_X_DMA", "sync")  # engine hosting x loads
XV_DMA = os.environ.get("K_XV_DMA", "")  # separate queue for the fp16 x stream
QKV_BUFS = int(os.environ.get("K_QKV_BUFS", "2"))
X_BUFS = int(os.environ.get("K_X_BUFS", "5"))


def _dtypes(mybir):
    """(x dtype for q/k proj, x dtype for v proj, matmul/probs dtype)."""
    f16 = mybir.dt.float16
    f8 = mybir.dt.float8e4
    if PROJ_DTYPE == "fp8":
        return f8, f8, mybir.dt.bfloat16
    if PROJ_DTYPE == "mixed":
        return f8, f16, f16
    if PROJ_DTYPE == "fp16":
        return f16, f16, f16
    return mybir.dt.bfloat16, mybir.dt.bfloat16, mybir.dt.bfloat16


def build_program(iters=1):
    global _PROGRAMS
    key = (
        iters, PHASES, PROJ_DTYPE, PS_CFG, PPOOL, OUT_DMA, OUT_DTYPE,
        X_DMA, XV_DMA, QKV_BUFS, X_BUFS, CAST_ON_DEVICE, CAST_ENG,
        VTRANS, VTRANS_DMA,
    )
    if key in _PROGRAMS:
        return _PROGRAMS[key]

    import concourse.bacc as bacc
    import concourse.mybir as mybir
    import concourse.tile as tile
    from concourse.masks import make_identity

    f32 = mybir.dt.float32

    nc = bacc.Bacc("TRN2", target_bir_lowering=False, debug=False)

    qk_dt, v_dt, mmdt = _dtypes(mybir)
    split_x = qk_dt != v_dt
    ship_x8 = split_x and not CAST_ON_DEVICE
    odt = mybir.dt.float16 if OUT_DTYPE == "fp16" else f32

    # q/k/v weights and biases arrive host-packed into single tensors in
    # the exact SBUF tile layout
    if split_x:
        xv_d = nc.dram_tensor("xT16", [E, S], v_dt, kind="ExternalInput")
        xq_d = (
            nc.dram_tensor("xT8", [E, S], qk_dt, kind="ExternalInput")
            if ship_x8
            else None
        )
        wqk_d = nc.dram_tensor("wqk", [P, 2 * NE * D], qk_dt, kind="ExternalInput")
        wv_d = nc.dram_tensor("wv", [P, NE * D], v_dt, kind="ExternalInput")
    else:
        xq_d = xv_d = nc.dram_tensor("xT", [E, S], qk_dt, kind="ExternalInput")
        wqk_d = nc.dram_tensor("wqkv", [P, 3 * NE * D], qk_dt, kind="ExternalInput")
        wv_d = None
    bias_d = nc.dram_tensor("bias", [D, 3], f32, kind="ExternalInput")
    out_d = nc.dram_tensor("out", [S, D], odt, kind="ExternalOutput")

    with tile.TileContext(nc) as tc:
        with (
            tc.tile_pool(name="const", bufs=1) as cpool,
            tc.tile_pool(name="xq", bufs=X_BUFS) as xqpool,
            tc.tile_pool(name="xv", bufs=X_BUFS) as xvpool,
            tc.tile_pool(name="qkv", bufs=QKV_BUFS) as qkvpool,
            tc.tile_pool(name="probs", bufs=PPOOL) as ppool,
            tc.tile_pool(name="osb", bufs=2) as opool,
            tc.tile_pool(name="misc", bufs=2) as mpool,
            tc.tile_pool(name="proj_ps", bufs=PS_CFG[0], space="PSUM") as proj_ps,
            tc.tile_pool(name="sc_ps", bufs=PS_CFG[1], space="PSUM") as sc_ps,
            tc.tile_pool(name="vt_ps", bufs=1, space="PSUM") as vt_ps,
            tc.tile_pool(name="out_ps", bufs=PS_CFG[2], space="PSUM") as out_ps,
        ):
            # ---- iteration-invariant setup ----
            ident = cpool.tile([P, P], mmdt, tag="ident")
            make_identity(nc, ident[:])
            # cmaskT[k_local, q_local]: 0 where q >= k (valid), -1e30 where q < k
            cmaskT = cpool.tile([P, P], f32, tag="cmaskT")
            nc.gpsimd.memset(cmaskT[:], 0.0)
            nc.gpsimd.affine_select(
                out=cmaskT[:],
                in_=cmaskT[:],
                compare_op=mybir.AluOpType.is_ge,
                fill=NEG,
                base=0,
                # iota[r, c] = c - r ; keep (0.0) where c - r >= 0
                pattern=[[1, P]],
                channel_multiplier=-1,
            )

            w_sb = {}
            if split_x:
                wqk_sb = cpool.tile([P, 2 * NE * D], qk_dt, tag="wqk")
                nc.sync.dma_start(wqk_sb[:], wqk_d[:, :])
                wv_sb = cpool.tile([P, NE * D], v_dt, tag="wv")
                nc.sync.dma_start(wv_sb[:], wv_d[:, :])
                w_sb["q"] = wqk_sb[:, 0 : NE * D]
                w_sb["k"] = wqk_sb[:, NE * D : 2 * NE * D]
                w_sb["v"] = wv_sb[:]
            else:
                wqkv_sb = cpool.tile([P, 3 * NE * D], qk_dt, tag="wqkv")
                nc.sync.dma_start(wqkv_sb[:], wqk_d[:, :])
                for n, pj in enumerate(("q", "k", "v")):
                    w_sb[pj] = wqkv_sb[:, n * NE * D : (n + 1) * NE * D]
            bias_sb = cpool.tile([P, 3], f32, tag="bias")
            nc.sync.dma_start(bias_sb[:], bias_d[:, :])
            b_sb = {pj: bias_sb[:, n : n + 1] for n, pj in enumerate(("q", "k", "v"))}

            # HAM warmup while the first DMAs land (runs once, cold)
            wps = proj_ps.tile([P, ST], f32, name="warm", tag="proj")
            for wi in range(40):
                nc.tensor.matmul(
                    wps[:, 0:P],
                    lhsT=ident[:],
                    rhs=ident[:],
                    start=(wi == 0),
                    stop=(wi == 39),
                )

            def body():
                _emit_body(
                    nc,
                    mybir,
                    pools={
                        "xqpool": xqpool,
                        "xvpool": xvpool,
                        "qkvpool": qkvpool,
                        "ppool": ppool,
                        "opool": opool,
                        "mpool": mpool,
                        "proj_ps": proj_ps,
                        "sc_ps": sc_ps,
                        "vt_ps": vt_ps,
                        "out_ps": out_ps,
                    },
                    dram={"xq": xq_d, "xv": xv_d, "out": out_d},
                    consts={
                        "ident": ident,
                        "cmaskT": cmaskT,
                        "w": w_sb,
                        "b": b_sb,
                    },
                )

            # unrolling the loop body halves the per-iteration all-engine
            # barrier count (the For_i reset barrier blocks cross-iteration
            # DMA prefetch and drains the PE pipeline)
            unroll = int(os.environ.get("K_UNROLL", "4"))
            if iters > 1:
                n_loop = iters // unroll
                rem = iters - n_loop * unroll
                hints = (
                    mybir.EngineType.PE,
                    mybir.EngineType.Activation,
                    mybir.EngineType.DVE,
                    mybir.EngineType.SP,
                    mybir.EngineType.Pool,
                )
                if n_loop > 1:
                    with tc.For_i(0, n_loop, 1, hint_engines=hints):
                        for _ in range(unroll):
                            body()
                else:
                    rem = iters
                for _ in range(rem):
                    body()
            else:
                body()

    nc.compile()
    _PROGRAMS[key] = nc
    return nc


def _emit_body(nc, mybir, pools, dram, consts):
    f32 = mybir.dt.float32
    qk_dt, v_dt, mmdt = _dtypes(mybir)
    split_x = qk_dt != v_dt
    ship_x8 = split_x and not CAST_ON_DEVICE
    fp8 = mybir.dt.float8e4

    xqpool = pools["xqpool"]
    xvpool = pools["xvpool"]
    qkvpool = pools["qkvpool"]
    ppool = pools["ppool"]
    opool = pools["opool"]
    mpool = pools["mpool"]
    proj_ps = pools["proj_ps"]
    sc_ps = pools["sc_ps"]
    vt_ps = pools["vt_ps"]
    out_ps = pools["out_ps"]
    xq_d = dram["xq"]
    xv_d = dram["xv"]
    out_d = dram["out"]
    ident = consts["ident"]
    cmaskT = consts["cmaskT"]
    w_sb = consts["w"]
    b_sb = consts["b"]

    x_eng = getattr(nc, X_DMA)
    xv_eng = getattr(nc, XV_DMA) if XV_DMA else x_eng
    cast_engs = [getattr(nc, e) for e in CAST_ENG.split(",")]

    # ---- xT loads: one tile per s-tile, prefetchable across iterations ----
    xv_v = xv_d.rearrange("(ec p) s -> p ec s", p=P)
    xq_v = xq_d.rearrange("(ec p) s -> p ec s", p=P) if ship_x8 else None
    xq_st = []
    xv_st = []
    for st in range(NST):
        xt16 = xvpool.tile([P, NE * ST], v_dt, name=f"xv{st}", tag="xv")
        if st == 0 and os.environ.get("K_FINE", "1") == "1":
            # fine-grained pieces: the first projection chain can start
            # after ~1 piece instead of waiting for the full tile
            for ec in range(NE):
                xv_eng.dma_start(
                    xt16[:, ec * ST : (ec + 1) * ST],
                    xv_v[:, ec, st * ST : (st + 1) * ST],
                )
        else:
            xv_eng.dma_start(
                xt16[:].rearrange("p (ec s) -> p ec s", ec=NE),
                xv_v[:, :, st * ST : (st + 1) * ST],
            )
        xv_st.append(xt16)
        if not split_x:
            xq_st.append(xt16)
        elif ship_x8:
            xt8 = xqpool.tile([P, NE * ST], qk_dt, name=f"xq{st}", tag="xq")
            if st == 0:
                for ec in range(NE):
                    x_eng.dma_start(
                        xt8[:, ec * ST : (ec + 1) * ST],
                        xq_v[:, ec, st * ST : (st + 1) * ST],
                    )
            else:
                x_eng.dma_start(
                    xt8[:].rearrange("p (ec s) -> p ec s", ec=NE),
                    xq_v[:, :, st * ST : (st + 1) * ST],
                )
            xq_st.append(xt8)
        else:
            # device-side fp16 -> fp8 cast: tiles allocated here, the cast
            # ops themselves are emitted just-in-time inside the s-tile
            # loop so they don't serialize ahead of the PSUM evacuations
            # in the in-order engine queues
            xt8 = xqpool.tile([P, NE * ST], fp8, name=f"xq{st}", tag="xq")
            xq_st.append(xt8)

    def emit_casts(st):
        """fp16 -> fp8 casts for tile st, pairs rotated across engines."""
        if not (split_x and not ship_x8):
            return
        for g in range(NE // 2):
            eng = cast_engs[g % len(cast_engs)]
            dst = xq_st[st][:, 2 * g * ST : (2 * g + 2) * ST]
            src = xv_st[st][:, 2 * g * ST : (2 * g + 2) * ST]
            if hasattr(eng, "tensor_copy"):
                eng.tensor_copy(dst, src)
            else:
                eng.copy(dst, src)

    qT_sb = qkvpool.tile([P, S], mmdt, tag="qT")
    kT_sb = qkvpool.tile([P, S], mmdt, tag="kT")
    vT_sb = qkvpool.tile([P, S], mmdt, tag="vT")
    v_sb = qkvpool.tile([P, NS * VSTRIDE], mmdt, tag="v")
    dest = {"q": qT_sb, "k": kT_sb, "v": vT_sb}

    # ones column of v_aug
    for sb in range(NS):
        nc.vector.memset(v_sb[:, sb * VSTRIDE + D : sb * VSTRIDE + D + 1], 1.0)

    def emit_proj(pj, st):
        """Project chunk st of x onto head dim for q/k/v; write dest[pj]."""
        x_sb = xq_st[st] if pj in ("q", "k") else xv_st[st]
        xdt = qk_dt if pj in ("q", "k") else v_dt
        ps = proj_ps.tile([P, ST], f32, tag="proj")
        if xdt == fp8:
            for g in range(NE // 2):
                nc.tensor.matmul(
                    ps[:],
                    lhsT=w_sb[pj][:, 2 * g * D : (2 * g + 2) * D].rearrange(
                        "p (i d) -> p i d", i=2
                    ),
                    rhs=x_sb[:, 2 * g * ST : (2 * g + 2) * ST].rearrange(
                        "p (i s) -> p i s", i=2
                    ),
                    start=(g == 0),
                    stop=(g == NE // 2 - 1),
                    perf_mode=mybir.MatmulPerfMode.DoubleRow,
                )
            nc.vector.tensor_scalar(
                dest[pj][:, st * ST : (st + 1) * ST],
                ps[:],
                1.0 / W_SCALE,
                b_sb[pj],
                op0=mybir.AluOpType.mult,
                op1=mybir.AluOpType.add,
            )
        else:
            for ec in range(NE):
                nc.tensor.matmul(
                    ps[:],
                    lhsT=w_sb[pj][:, ec * D : (ec + 1) * D],
                    rhs=x_sb[:, ec * ST : (ec + 1) * ST],
                    start=(ec == 0),
                    stop=(ec == NE - 1),
                )
            nc.vector.tensor_scalar_add(
                dest[pj][:, st * ST : (st + 1) * ST],
                ps[:],
                b_sb[pj],
            )

    probs_pieces = {}

    def emit_piece(c, j):
        qs = max(c * ST, j * P)
        w = (c + 1) * ST - qs
        sps = sc_ps.tile([P, ST], f32, tag="sc")
        nc.tensor.matmul(
            sps[:, :w],
            lhsT=kT_sb[:, j * P : (j + 1) * P],
            rhs=qT_sb[:, qs : qs + w],
            start=True,
            stop=True,
        )
        if j * P >= c * ST:
            nc.vector.tensor_add(sps[:, 0:P], sps[:, 0:P], cmaskT[:])
        prb = ppool.tile([P, ST], mmdt, name="prb", tag="probs")
        nc.scalar.activation(
            prb[:, :w],
            sps[:, :w],
            func=mybir.ActivationFunctionType.Exp,
            bias=0.0,
            scale=SCALE,
        )
        probs_pieces[(j, c)] = (prb, qs)

    odt = mybir.dt.float16 if OUT_DTYPE == "fp16" else f32
    out_v = out_d.rearrange("(c i p) d -> p c i d", p=P, i=ST // P)
    for st in range(NST):
        # casts for tile 0 must precede its projections; casts for later
        # tiles are emitted one tile ahead (end of the previous section)
        if st == 0:
            emit_casts(0)

        # ---- projections for this s-tile ----
        # order q, k first so the score/exp stream for this chunk can
        # start while the v projection and transposes still run
        for pj in ("q", "k") if "proj" in PHASES else ():
            emit_proj(pj, st)

        # ---- scoresT + exp for q-chunk c = st, interleaved with the v
        # projection: the score stream is paced by the ScalarE exp
        # evacuations (~3x slower than the PE matmul per piece), so the
        # independent v-projection chunks keep the PE busy in between ----
        c = st
        npieces = 4 * c + 4 if "scores" in PHASES else 0
        x_sb = xv_st[st]
        vps = None
        if "proj" in PHASES:
            vps = proj_ps.tile([P, ST], f32, name="vps", tag="proj")
        nv = NE if "proj" in PHASES else 0
        vdone = 0
        for j in range(npieces):
            emit_piece(c, j)
            vtarget = (j + 1) * nv // npieces
            while vdone < vtarget:
                ec = vdone
                nc.tensor.matmul(
                    vps[:],
                    lhsT=w_sb["v"][:, ec * D : (ec + 1) * D],
                    rhs=x_sb[:, ec * ST : (ec + 1) * ST],
                    start=(ec == 0),
                    stop=(ec == NE - 1),
                    skip_group_check=True,
                )
                vdone += 1
        if nv:
            while vdone < nv:
                ec = vdone
                nc.tensor.matmul(
                    vps[:],
                    lhsT=w_sb["v"][:, ec * D : (ec + 1) * D],
                    rhs=x_sb[:, ec * ST : (ec + 1) * ST],
                    start=(ec == 0),
                    stop=(ec == NE - 1),
                    skip_group_check=True,
                )
                vdone += 1
            nc.vector.tensor_scalar_add(
                vT_sb[:, st * ST : (st + 1) * ST],
                vps[:],
                b_sb["v"],
            )

        # prefetch-cast the next tile while this tile's scores/AV run
        if st + 1 < NST:
            emit_casts(st + 1)

        # ---- v natural blocks (PE transpose of vT, or DMA transpose) ----
        for sb in (
            range(st * (ST // P), (st + 1) * (ST // P)) if "vtrans" in PHASES else ()
        ):
            if VTRANS == "dma":
                getattr(nc, VTRANS_DMA).dma_start_transpose(
                    v_sb[:, sb * VSTRIDE : sb * VSTRIDE + D],
                    vT_sb[:, sb * P : (sb + 1) * P],
                )
            else:
                tp = vt_ps.tile([P, P], mmdt, tag="vt")
                nc.tensor.transpose(tp[:], vT_sb[:, sb * P : (sb + 1) * P], ident[:])
                nc.vector.tensor_copy(v_sb[:, sb * VSTRIDE : sb * VSTRIDE + D], tp[:])

        # ---- AV + normalize + store for the 4 q-blocks of chunk c ----
        osb = opool.tile([P, (ST // P) * D], odt, tag="osb")
        for i in range(4 * c, 4 * c + 4) if "av" in PHASES else ():
            ops = out_ps.tile([P, D + 1], f32, tag="out")
            for j in range(i + 1):
                prb, qs = probs_pieces[(j, c)]
                off = i * P - qs
                nc.tensor.matmul(
                    ops[:],
                    lhsT=prb[:, off : off + P],
                    rhs=v_sb[:, j * VSTRIDE : j * VSTRIDE + VW],
                    start=(j == 0),
                    stop=(j == i),
                )
            recip = mpool.tile([P, 1], f32, tag="recip")
            nc.vector.reciprocal(recip[:], ops[:, D : D + 1])
            il = i - 4 * c
            nc.vector.tensor_scalar_mul(
                osb[:, il * D : (il + 1) * D], ops[:, 0:D], recip[:, 0:1]
            )
        # one batched store for the whole 512-row chunk
        if "av" in PHASES:
            eng = getattr(nc, OUT_DMA)
            eng.dma_start(
                out_v[:, c, :, :],
                osb[:].rearrange("p (i d) -> p i d", i=ST // P),
            )


def make_in_maps(x, Wq, bq, Wk, bk, Wv, bv):
    import ml_dtypes

    fp8 = ml_dtypes.float8_e4m3
    f16 = np.float16
    bf16 = ml_dtypes.bfloat16
    x = np.asarray(x, dtype=np.float32)

    def wcast(W, dt, scale):
        wt = np.asarray(W, dtype=np.float32).T * scale  # [E, D]
        packed = wt.reshape(NE, P, D).transpose(1, 0, 2).reshape(P, NE * D)
        return np.ascontiguousarray(packed).astype(dt)

    bias = np.ascontiguousarray(
        np.stack([np.asarray(b, dtype=np.float32) for b in (bq, bk, bv)], axis=1)
    )
    if PROJ_DTYPE == "mixed":
        shared = {
            "wqk": np.ascontiguousarray(
                np.concatenate(
                    [wcast(Wq, fp8, W_SCALE), wcast(Wk, fp8, W_SCALE)], axis=1
                )
            ),
            "wv": wcast(Wv, f16, 1.0),
            "bias": bias,
        }
        maps = []
        for b in range(B):
            m = {"xT16": np.ascontiguousarray(x[b].T).astype(f16), **shared}
            if not CAST_ON_DEVICE:
                m["xT8"] = np.ascontiguousarray(x[b].T).astype(fp8)
            maps.append(m)
        return maps

    if PROJ_DTYPE == "fp8":
        xdt, wscale = fp8, W_SCALE
    elif PROJ_DTYPE == "fp16":
        xdt, wscale = f16, 1.0
    else:
        xdt, wscale = bf16, 1.0
    shared = {
        "wqkv": np.ascontiguousarray(
            np.concatenate(
                [wcast(W, xdt, wscale) for W in (Wq, Wk, Wv)], axis=1
            )
        ),
        "bias": bias,
    }
    return [
        {"xT": np.ascontiguousarray(x[b].T).astype(xdt), **shared}
        for b in range(B)
    ]


def kernel(x, Wq, bq, Wk, bk, Wv, bv):
    from concourse.bass_utils import run_bass_kernel_spmd

    nc = build_program()
    in_maps = make_in_maps(x, Wq, bq, Wk, bk, Wv, bv)
    res = run_bass_kernel_spmd(nc, in_maps, list(range(B)))
    return np.stack(
        [res.results[i]["out"].astype(np.float32) for i in range(B)], axis=0
    )


# revision 24
# speedup vs baseline: 1.1592x; 1.1592x over previous
"""Single-head causal attention (B=8, S=2048, E=2048, D=128) on 8 trn2 cores.

Sharding: data-parallel over batch — one batch element per NeuronCore.

Host marshaling per core: xT = x[b].T cast to fp16; the q/k/v weights are
transposed, cast, and packed into single tensors in the exact SBUF tile
layout (contiguous line-rate DMAs), the three biases into ONE [128, 3]
tensor.

Projection precision modes (PROJ_DTYPE):
  - "fp16": all three projections fp16 (1 col/cycle PE rate)
  - "fp8":  all three projections fp8e4m3 DoubleRow (2x PE rate, v-path
            quantization error ~3e-2 — fails tight tolerances)
  - "mixed" (default): q,k projections fp8 DoubleRow, v projection fp16.
            The fp8 error only perturbs softmax logits, which are scaled
            by 1/sqrt(2048) — output error stays ~4e-3 while 2/3 of the
            projection FLOPs run at 2x rate. With CAST_ON_DEVICE (default)
            the fp8 copy of x is produced by the DVE from the fp16 stream
            (hidden under the DMA), so HBM traffic stays at the fp16
            baseline; otherwise xT ships twice (fp8 + fp16).

Per-core dataflow (f32 PSUM accumulation):
  - projections produce qT/kT/vT in [D, S] layout; bias added during the
    VectorE PSUM->SBUF evacuation (per-partition scalar add)
  - vT is re-transposed on the PE into natural [S, D] blocks, augmented
    with a ones column (col 128): the AV matmul then yields the softmax
    denominator for free as output column 128
  - scoresT[k, q] per k-block j: single matmul (K=D=128), exact causal
    trim of the q range; diagonal 128-block masked by adding -1e30;
    ScalarE computes exp(scale*s) straight out of PSUM into fp16 probsT.
    The score stream is paced by these ScalarE evacuations (~3x the PE
    cost per piece), so the independent v-projection chunks are
    interleaved between score pieces to keep the in-order PE queue fed
  - AV per q-block i accumulates probsT_j.T @ v_aug_j over j<=i in PSUM;
    VectorE takes 1/denominator and applies it during the final
    evacuation; one batched DMA per 512-row chunk stores fp16 output
    (host upcasts to f32)

Loop-timing structure: the For_i body is unrolled 4x — each For_i
iteration carries an all-engine barrier in its reset block (~6us of
drain + pipeline/DMA-prefetch refill), so fewer, fatter iterations
amortize it. qT/kT/vT/v and x tiles are multi-buffered so consecutive
bodies overlap (next body's DMA/casts/projections run under the current
body's scores/AV tail).
"""

import math
import os

import numpy as np

B = 8
S = 2048
E = 2048
D = 128
P = 128
NE = E // P  # 16 contraction chunks
NS = S // P  # 16 sequence blocks
ST = 512  # s-tile width for projections / score chunks
NST = S // ST  # 4
VW = D + 1  # logical v block width incl. ones column
VSTRIDE = D + 1  # physical stride of v blocks in SBUF
SCALE = 1.0 / math.sqrt(S)
NEG = -1.0e30
NEG16 = -60000.0  # fp16-representable; exp underflows to exactly 0

_PROGRAMS = {}

# which phases to emit (for microbenchmarking): subset of
# {"proj", "vtrans", "scores", "av", "store"}
PHASES = frozenset(
    p
    for p in os.environ.get("K_PHASES", "proj,vtrans,scores,av,store").split(",")
    if p
)

# per-projection matmul precision; see module docstring
PROJ_DTYPE = os.environ.get("K_PROJ_DTYPE", "mixed")
W_SCALE = 256.0  # host pre-scale of W before fp8 quantization (2**8: exact)
# produce the fp8 x copy on-device (Pool engine) instead of shipping it
CAST_ON_DEVICE = os.environ.get("K_CAST", "1") == "1"
# comma-separated rotation of engines hosting the cast ops
CAST_ENG = os.environ.get("K_CAST_ENG", "vector")

# tunables: PSUM bank split (proj, sc, out; vt fixed at 1) and probs pool depth
PS_CFG = tuple(int(v) for v in os.environ.get("K_PS_CFG", "3,2,2").split(","))
# v natural-block transpose: "pe" (tensor-engine transpose + DVE copy) or
# "dma" (SBUF->SBUF DMA transpose, frees PE/DVE/PSUM)
VTRANS = os.environ.get("K_VTRANS", "pe")
VTRANS_DMA = os.environ.get("K_VTRANS_DMA", "sync")
PPOOL = int(os.environ.get("K_PPOOL", "24"))
OUT_DMA = os.environ.get("K_OUT_DMA", "gpsimd")  # engine hosting output stores
OUT_DTYPE = os.environ.get("K_OUT_DTYPE", "fp16")  # f32|fp16 store dtype
X_DMA = os.environ.get("K_X_DMA", "sync")  # engine hosting x loads
XV_DMA = os.environ.get("K_XV_DMA", "")  # separate queue for the fp16 x stream
QKV_BUFS = int(os.environ.get("K_QKV_BUFS", "2"))
X_BUFS = int(os.environ.get("K_X_BUFS", "5"))
# causal mask of the diagonal score block: "pe" accumulates an
# identity-matmul of the fp16 mask into the score PSUM (keeps the exp
# critical path off the DVE); "dve" adds the f32 mask on the VectorE
MASK_ON_PE = os.environ.get("K_MASK", "pe")


def _dtypes(mybir):
    """(x dtype for q/k proj, x dtype for v proj, matmul/probs dtype)."""
    f16 = mybir.dt.float16
    f8 = mybir.dt.float8e4
    if PROJ_DTYPE == "fp8":
        return f8, f8, mybir.dt.bfloat16
    if PROJ_DTYPE == "mixed":
        return f8, f16, f16
    if PROJ_DTYPE == "fp16":
        return f16, f16, f16
    return mybir.dt.bfloat16, mybir.dt.bfloat16, mybir.dt.bfloat16


def build_program(iters=1):
    global _PROGRAMS
    key = (
        iters, PHASES, PROJ_DTYPE, PS_CFG, PPOOL, OUT_DMA, OUT_DTYPE,
        X_DMA, XV_DMA, QKV_BUFS, X_BUFS, CAST_ON_DEVICE, CAST_ENG,
        VTRANS, VTRANS_DMA, MASK_ON_PE,
    )
    if key in _PROGRAMS:
        return _PROGRAMS[key]

    import concourse.bacc as bacc
    import concourse.mybir as mybir
    import concourse.tile as tile
    from concourse.masks import make_identity

    f32 = mybir.dt.float32

    nc = bacc.Bacc("TRN2", target_bir_lowering=False, debug=False)

    qk_dt, v_dt, mmdt = _dtypes(mybir)
    split_x = qk_dt != v_dt
    ship_x8 = split_x and not CAST_ON_DEVICE
    odt = mybir.dt.float16 if OUT_DTYPE == "fp16" else f32

    # q/k/v weights and biases arrive host-packed into single tensors in
    # the exact SBUF tile layout
    if split_x:
        xv_d = nc.dram_tensor("xT16", [E, S], v_dt, kind="ExternalInput")
        xq_d = (
            nc.dram_tensor("xT8", [E, S], qk_dt, kind="ExternalInput")
            if ship_x8
            else None
        )
        wqk_d = nc.dram_tensor("wqk", [P, 2 * NE * D], qk_dt, kind="ExternalInput")
        wv_d = nc.dram_tensor("wv", [P, NE * D], v_dt, kind="ExternalInput")
    else:
        xq_d = xv_d = nc.dram_tensor("xT", [E, S], qk_dt, kind="ExternalInput")
        wqk_d = nc.dram_tensor("wqkv", [P, 3 * NE * D], qk_dt, kind="ExternalInput")
        wv_d = None
    bias_d = nc.dram_tensor("bias", [D, 3], f32, kind="ExternalInput")
    out_d = nc.dram_tensor("out", [S, D], odt, kind="ExternalOutput")

    with tile.TileContext(nc) as tc:
        with (
            tc.tile_pool(name="const", bufs=1) as cpool,
            tc.tile_pool(name="xq", bufs=X_BUFS) as xqpool,
            tc.tile_pool(name="xv", bufs=X_BUFS) as xvpool,
            tc.tile_pool(name="qkv", bufs=QKV_BUFS) as qkvpool,
            tc.tile_pool(name="probs", bufs=PPOOL) as ppool,
            tc.tile_pool(name="osb", bufs=int(os.environ.get("K_OSB_BUFS", "2"))) as opool,
            tc.tile_pool(name="misc", bufs=2) as mpool,
            tc.tile_pool(name="proj_ps", bufs=PS_CFG[0], space="PSUM") as proj_ps,
            tc.tile_pool(name="sc_ps", bufs=PS_CFG[1], space="PSUM") as sc_ps,
            tc.tile_pool(name="vt_ps", bufs=1, space="PSUM") as vt_ps,
            tc.tile_pool(name="out_ps", bufs=PS_CFG[2], space="PSUM") as out_ps,
        ):
            # ---- iteration-invariant setup ----
            ident = cpool.tile([P, P], mmdt, tag="ident")
            make_identity(nc, ident[:])
            # cmaskT[k_local, q_local]: 0 where q >= k (valid), -1e30 where q < k
            cm_dt = f32 if MASK_ON_PE == "dve" else mmdt
            cm_fill = NEG if MASK_ON_PE == "dve" else NEG16
            cmaskT = cpool.tile([P, P], cm_dt, tag="cmaskT")
            nc.gpsimd.memset(cmaskT[:], 0.0)
            nc.gpsimd.affine_select(
                out=cmaskT[:],
                in_=cmaskT[:],
                compare_op=mybir.AluOpType.is_ge,
                fill=cm_fill,
                base=0,
                # iota[r, c] = c - r ; keep (0.0) where c - r >= 0
                pattern=[[1, P]],
                channel_multiplier=-1,
            )

            w_sb = {}
            if split_x:
                wqk_sb = cpool.tile([P, 2 * NE * D], qk_dt, tag="wqk")
                nc.sync.dma_start(wqk_sb[:], wqk_d[:, :])
                wv_sb = cpool.tile([P, NE * D], v_dt, tag="wv")
                nc.sync.dma_start(wv_sb[:], wv_d[:, :])
                w_sb["q"] = wqk_sb[:, 0 : NE * D]
                w_sb["k"] = wqk_sb[:, NE * D : 2 * NE * D]
                w_sb["v"] = wv_sb[:]
            else:
                wqkv_sb = cpool.tile([P, 3 * NE * D], qk_dt, tag="wqkv")
                nc.sync.dma_start(wqkv_sb[:], wqk_d[:, :])
                for n, pj in enumerate(("q", "k", "v")):
                    w_sb[pj] = wqkv_sb[:, n * NE * D : (n + 1) * NE * D]
            bias_sb = cpool.tile([P, 3], f32, tag="bias")
            nc.sync.dma_start(bias_sb[:], bias_d[:, :])
            b_sb = {pj: bias_sb[:, n : n + 1] for n, pj in enumerate(("q", "k", "v"))}

            # HAM warmup while the first DMAs land (runs once, cold)
            wps = proj_ps.tile([P, ST], f32, name="warm", tag="proj")
            for wi in range(40):
                nc.tensor.matmul(
                    wps[:, 0:P],
                    lhsT=ident[:],
                    rhs=ident[:],
                    start=(wi == 0),
                    stop=(wi == 39),
                )

            def body():
                _emit_body(
                    nc,
                    mybir,
                    pools={
                        "xqpool": xqpool,
                        "xvpool": xvpool,
                        "qkvpool": qkvpool,
                        "ppool": ppool,
                        "opool": opool,
                        "mpool": mpool,
                        "proj_ps": proj_ps,
                        "sc_ps": sc_ps,
                        "vt_ps": vt_ps,
                        "out_ps": out_ps,
                    },
                    dram={"xq": xq_d, "xv": xv_d, "out": out_d},
                    consts={
                        "ident": ident,
                        "cmaskT": cmaskT,
                        "w": w_sb,
                        "b": b_sb,
                    },
                )

            # unrolling the loop body halves the per-iteration all-engine
            # barrier count (the For_i reset barrier blocks cross-iteration
            # DMA prefetch and drains the PE pipeline)
            unroll = int(os.environ.get("K_UNROLL", "4"))
            if iters > 1:
                n_loop = iters // unroll
                rem = iters - n_loop * unroll
                hints = (
                    mybir.EngineType.PE,
                    mybir.EngineType.Activation,
                    mybir.EngineType.DVE,
                    mybir.EngineType.SP,
                    mybir.EngineType.Pool,
                )
                if n_loop > 1:
                    with tc.For_i(0, n_loop, 1, hint_engines=hints):
                        for _ in range(unroll):
                            body()
                else:
                    rem = iters
                for _ in range(rem):
                    body()
            else:
                body()

    nc.compile()
    _PROGRAMS[key] = nc
    return nc


def _emit_body(nc, mybir, pools, dram, consts):
    f32 = mybir.dt.float32
    qk_dt, v_dt, mmdt = _dtypes(mybir)
    split_x = qk_dt != v_dt
    ship_x8 = split_x and not CAST_ON_DEVICE
    fp8 = mybir.dt.float8e4

    xqpool = pools["xqpool"]
    xvpool = pools["xvpool"]
    qkvpool = pools["qkvpool"]
    ppool = pools["ppool"]
    opool = pools["opool"]
    mpool = pools["mpool"]
    proj_ps = pools["proj_ps"]
    sc_ps = pools["sc_ps"]
    vt_ps = pools["vt_ps"]
    out_ps = pools["out_ps"]
    xq_d = dram["xq"]
    xv_d = dram["xv"]
    out_d = dram["out"]
    ident = consts["ident"]
    cmaskT = consts["cmaskT"]
    w_sb = consts["w"]
    b_sb = consts["b"]

    x_eng = getattr(nc, X_DMA)
    xv_eng = getattr(nc, XV_DMA) if XV_DMA else x_eng
    cast_engs = [getattr(nc, e) for e in CAST_ENG.split(",")]

    # ---- xT loads: one tile per s-tile, prefetchable across iterations ----
    xv_v = xv_d.rearrange("(ec p) s -> p ec s", p=P)
    xq_v = xq_d.rearrange("(ec p) s -> p ec s", p=P) if ship_x8 else None
    xq_st = []
    xv_st = []
    for st in range(NST):
        xt16 = xvpool.tile([P, NE * ST], v_dt, name=f"xv{st}", tag="xv")
        if st == 0 and os.environ.get("K_FINE", "1") == "1":
            # fine-grained pieces: the first projection chain can start
            # after ~1 piece instead of waiting for the full tile
            for ec in range(NE):
                xv_eng.dma_start(
                    xt16[:, ec * ST : (ec + 1) * ST],
                    xv_v[:, ec, st * ST : (st + 1) * ST],
                )
        else:
            xv_eng.dma_start(
                xt16[:].rearrange("p (ec s) -> p ec s", ec=NE),
                xv_v[:, :, st * ST : (st + 1) * ST],
            )
        xv_st.append(xt16)
        if not split_x:
            xq_st.append(xt16)
        elif ship_x8:
            xt8 = xqpool.tile([P, NE * ST], qk_dt, name=f"xq{st}", tag="xq")
            if st == 0:
                for ec in range(NE):
                    x_eng.dma_start(
                        xt8[:, ec * ST : (ec + 1) * ST],
                        xq_v[:, ec, st * ST : (st + 1) * ST],
                    )
            else:
                x_eng.dma_start(
                    xt8[:].rearrange("p (ec s) -> p ec s", ec=NE),
                    xq_v[:, :, st * ST : (st + 1) * ST],
                )
            xq_st.append(xt8)
        else:
            # device-side fp16 -> fp8 cast: tiles allocated here, the cast
            # ops themselves are emitted just-in-time inside the s-tile
            # loop so they don't serialize ahead of the PSUM evacuations
            # in the in-order engine queues
            xt8 = xqpool.tile([P, NE * ST], fp8, name=f"xq{st}", tag="xq")
            xq_st.append(xt8)

    def emit_casts(st):
        """fp16 -> fp8 casts for tile st, pairs rotated across engines."""
        if not (split_x and not ship_x8):
            return
        for g in range(NE // 2):
            eng = cast_engs[g % len(cast_engs)]
            dst = xq_st[st][:, 2 * g * ST : (2 * g + 2) * ST]
            src = xv_st[st][:, 2 * g * ST : (2 * g + 2) * ST]
            if hasattr(eng, "tensor_copy"):
                eng.tensor_copy(dst, src)
            else:
                eng.copy(dst, src)

    qT_sb = qkvpool.tile([P, S], mmdt, tag="qT")
    kT_sb = qkvpool.tile([P, S], mmdt, tag="kT")
    vT_sb = qkvpool.tile([P, S], mmdt, tag="vT")
    v_sb = qkvpool.tile([P, NS * VSTRIDE], mmdt, tag="v")
    dest = {"q": qT_sb, "k": kT_sb, "v": vT_sb}

    # ones column of v_aug
    for sb in range(NS):
        nc.vector.memset(v_sb[:, sb * VSTRIDE + D : sb * VSTRIDE + D + 1], 1.0)

    def emit_proj(pj, st):
        """Project chunk st of x onto head dim for q/k/v; write dest[pj]."""
        x_sb = xq_st[st] if pj in ("q", "k") else xv_st[st]
        xdt = qk_dt if pj in ("q", "k") else v_dt
        ps = proj_ps.tile([P, ST], f32, tag="proj")
        if xdt == fp8:
            for g in range(NE // 2):
                nc.tensor.matmul(
                    ps[:],
                    lhsT=w_sb[pj][:, 2 * g * D : (2 * g + 2) * D].rearrange(
                        "p (i d) -> p i d", i=2
                    ),
                    rhs=x_sb[:, 2 * g * ST : (2 * g + 2) * ST].rearrange(
                        "p (i s) -> p i s", i=2
                    ),
                    start=(g == 0),
                    stop=(g == NE // 2 - 1),
                    perf_mode=mybir.MatmulPerfMode.DoubleRow,
                )
            nc.vector.tensor_scalar(
                dest[pj][:, st * ST : (st + 1) * ST],
                ps[:],
                1.0 / W_SCALE,
                b_sb[pj],
                op0=mybir.AluOpType.mult,
                op1=mybir.AluOpType.add,
            )
        else:
            for ec in range(NE):
                nc.tensor.matmul(
                    ps[:],
                    lhsT=w_sb[pj][:, ec * D : (ec + 1) * D],
                    rhs=x_sb[:, ec * ST : (ec + 1) * ST],
                    start=(ec == 0),
                    stop=(ec == NE - 1),
                )
            nc.vector.tensor_scalar_add(
                dest[pj][:, st * ST : (st + 1) * ST],
                ps[:],
                b_sb[pj],
            )

    probs_pieces = {}

    def emit_piece(c, j):
        qs = max(c * ST, j * P)
        w = (c + 1) * ST - qs
        sps = sc_ps.tile([P, ST], f32, tag="sc")
        diag = j * P >= c * ST
        nc.tensor.matmul(
            sps[:, :w],
            lhsT=kT_sb[:, j * P : (j + 1) * P],
            rhs=qT_sb[:, qs : qs + w],
            start=True,
            stop=not (diag and MASK_ON_PE == "pe"),
            skip_group_check=True,
        )
        if diag:
            if MASK_ON_PE == "pe":
                nc.tensor.matmul(
                    sps[:, 0:P],
                    lhsT=ident[:],
                    rhs=cmaskT[:],
                    start=False,
                    stop=True,
                    skip_group_check=True,
                )
            else:
                nc.vector.tensor_add(sps[:, 0:P], sps[:, 0:P], cmaskT[:])
        prb = ppool.tile([P, ST], mmdt, name="prb", tag="probs")
        nc.scalar.activation(
            prb[:, :w],
            sps[:, :w],
            func=mybir.ActivationFunctionType.Exp,
            bias=0.0,
            scale=SCALE,
        )
        probs_pieces[(j, c)] = (prb, qs)

    odt = mybir.dt.float16 if OUT_DTYPE == "fp16" else f32
    out_v = out_d.rearrange("(c i p) d -> p c i d", p=P, i=ST // P)
    for st in range(NST):
        # casts for tile 0 must precede its projections; casts for later
        # tiles are emitted one tile ahead (end of the previous section)
        if st == 0:
            emit_casts(0)

        # ---- projections for this s-tile ----
        # order q, k first so the score/exp stream for this chunk can
        # start while the v projection and transposes still run
        for pj in ("q", "k") if "proj" in PHASES else ():
            emit_proj(pj, st)

        # ---- scoresT + exp for q-chunk c = st, interleaved with the v
        # projection: the score stream is paced by the ScalarE exp
        # evacuations (~3x slower than the PE matmul per piece), so the
        # independent v-projection chunks keep the PE busy in between ----
        c = st
        npieces = 4 * c + 4 if "scores" in PHASES else 0
        x_sb = xv_st[st]
        vps = None
        if "proj" in PHASES:
            vps = proj_ps.tile([P, ST], f32, name="vps", tag="proj")
        nv = NE if "proj" in PHASES else 0
        vdone = 0
        for j in range(npieces):
            emit_piece(c, j)
            vtarget = (j + 1) * nv // npieces
            while vdone < vtarget:
                ec = vdone
                nc.tensor.matmul(
                    vps[:],
                    lhsT=w_sb["v"][:, ec * D : (ec + 1) * D],
                    rhs=x_sb[:, ec * ST : (ec + 1) * ST],
                    start=(ec == 0),
                    stop=(ec == NE - 1),
                    skip_group_check=True,
                )
                vdone += 1
        if nv:
            while vdone < nv:
                ec = vdone
                nc.tensor.matmul(
                    vps[:],
                    lhsT=w_sb["v"][:, ec * D : (ec + 1) * D],
                    rhs=x_sb[:, ec * ST : (ec + 1) * ST],
                    start=(ec == 0),
                    stop=(ec == NE - 1),
                    skip_group_check=True,
                )
                vdone += 1
            nc.vector.tensor_scalar_add(
                vT_sb[:, st * ST : (st + 1) * ST],
                vps[:],
                b_sb["v"],
            )

        # prefetch-cast the next tile while this tile's scores/AV run
        if st + 1 < NST:
            emit_casts(st + 1)

        # ---- v natural blocks (PE transpose of vT, or DMA transpose) ----
        for sb in (
            range(st * (ST // P), (st + 1) * (ST // P)) if "vtrans" in PHASES else ()
        ):
            if VTRANS == "dma":
                getattr(nc, VTRANS_DMA).dma_start_transpose(
                    v_sb[:, sb * VSTRIDE : sb * VSTRIDE + D],
                    vT_sb[:, sb * P : (sb + 1) * P],
                )
            else:
                tp = vt_ps.tile([P, P], mmdt, tag="vt")
                nc.tensor.transpose(tp[:], vT_sb[:, sb * P : (sb + 1) * P], ident[:])
                nc.vector.tensor_copy(v_sb[:, sb * VSTRIDE : sb * VSTRIDE + D], tp[:])

        # ---- AV + normalize + store for the 4 q-blocks of chunk c ----
        osb = opool.tile([P, (ST // P) * D], odt, tag="osb")
        for i in range(4 * c, 4 * c + 4) if "av" in PHASES else ():
            ops = out_ps.tile([P, D + 1], f32, tag="out")
            for j in range(i + 1):
                prb, qs = probs_pieces[(j, c)]
                off = i * P - qs
                nc.tensor.matmul(
                    ops[:],
                    lhsT=prb[:, off : off + P],
                    rhs=v_sb[:, j * VSTRIDE : j * VSTRIDE + VW],
                    start=(j == 0),
                    stop=(j == i),
                )
            recip = mpool.tile([P, 1], f32, tag="recip")
            nc.vector.reciprocal(recip[:], ops[:, D : D + 1])
            il = i - 4 * c
            nc.vector.tensor_scalar_mul(
                osb[:, il * D : (il + 1) * D], ops[:, 0:D], recip[:, 0:1]
            )
        # one batched store for the whole 512-row chunk
        if "av" in PHASES:
            eng = getattr(nc, OUT_DMA)
            eng.dma_start(
                out_v[:, c, :, :],
                osb[:].rearrange("p (i d) -> p i d", i=ST // P),
            )


def make_in_maps(x, Wq, bq, Wk, bk, Wv, bv):
    import ml_dtypes

    fp8 = ml_dtypes.float8_e4m3
    f16 = np.float16
    bf16 = ml_dtypes.bfloat16
    x = np.asarray(x, dtype=np.float32)

    def wcast(W, dt, scale):
        wt = np.asarray(W, dtype=np.float32).T * scale  # [E, D]
        packed = wt.reshape(NE, P, D).transpose(1, 0, 2).reshape(P, NE * D)
        return np.ascontiguousarray(packed).astype(dt)

    bias = np.ascontiguousarray(
        np.stack([np.asarray(b, dtype=np.float32) for b in (bq, bk, bv)], axis=1)
    )
    if PROJ_DTYPE == "mixed":
        shared = {
            "wqk": np.ascontiguousarray(
                np.concatenate(
                    [wcast(Wq, fp8, W_SCALE), wcast(Wk, fp8, W_SCALE)], axis=1
                )
            ),
            "wv": wcast(Wv, f16, 1.0),
            "bias": bias,
        }
        maps = []
        for b in range(B):
            m = {"xT16": np.ascontiguousarray(x[b].T).astype(f16), **shared}
            if not CAST_ON_DEVICE:
                m["xT8"] = np.ascontiguousarray(x[b].T).astype(fp8)
            maps.append(m)
        return maps

    if PROJ_DTYPE == "fp8":
        xdt, wscale = fp8, W_SCALE
    elif PROJ_DTYPE == "fp16":
        xdt, wscale = f16, 1.0
    else:
        xdt, wscale = bf16, 1.0
    shared = {
        "wqkv": np.ascontiguousarray(
            np.concatenate(
                [wcast(W, xdt, wscale) for W in (Wq, Wk, Wv)], axis=1
            )
        ),
        "bias": bias,
    }
    return [
        {"xT": np.ascontiguousarray(x[b].T).astype(xdt), **shared}
        for b in range(B)
    ]


def kernel(x, Wq, bq, Wk, bk, Wv, bv):
    from concourse.bass_utils import run_bass_kernel_spmd

    nc = build_program()
    in_maps = make_in_maps(x, Wq, bq, Wk, bk, Wv, bv)
    res = run_bass_kernel_spmd(nc, in_maps, list(range(B)))
    return np.stack(
        [res.results[i]["out"].astype(np.float32) for i in range(B)], axis=0
    )


# revision 25
# speedup vs baseline: 1.3074x; 1.1278x over previous
"""Single-head causal attention (B=8, S=2048, E=2048, D=128) on 8 trn2 cores.

Sharding: data-parallel over batch — one batch element per NeuronCore.

Host marshaling per core: xT = x[b].T cast to fp16; the q/k/v weights are
transposed, cast, and packed into single tensors in the exact SBUF tile
layout (contiguous line-rate DMAs), the three biases into ONE [128, 3]
tensor.

Projection precision modes (PROJ_DTYPE):
  - "fp16": all three projections fp16 (1 col/cycle PE rate)
  - "fp8":  all three projections fp8e4m3 DoubleRow (2x PE rate, v-path
            quantization error ~3e-2 — fails tight tolerances)
  - "mixed" (default): q,k projections fp8 DoubleRow, v projection fp16.
            The fp8 error only perturbs softmax logits, which are scaled
            by 1/sqrt(2048) — output error stays ~4e-3 while 2/3 of the
            projection FLOPs run at 2x rate. With CAST_ON_DEVICE (default)
            the fp8 copy of x is produced by the DVE from the fp16 stream
            (hidden under the DMA), so HBM traffic stays at the fp16
            baseline; otherwise xT ships twice (fp8 + fp16).

Per-core dataflow (f32 PSUM accumulation):
  - projections produce qT/kT/vT in [D, S] layout; bias added during the
    VectorE PSUM->SBUF evacuation (per-partition scalar add)
  - vT is re-transposed on the PE into natural [S, D] blocks, augmented
    with a ones column (col 128): the AV matmul then yields the softmax
    denominator for free as output column 128
  - scoresT[k, q] per k-block j: single matmul (K=D=128), exact causal
    trim of the q range; diagonal 128-block masked by adding -1e30;
    ScalarE computes exp(scale*s) straight out of PSUM into fp16 probsT.
    The score stream is paced by these ScalarE evacuations (~3x the PE
    cost per piece), so the independent v-projection chunks are
    interleaved between score pieces to keep the in-order PE queue fed
  - AV per q-block i accumulates probsT_j.T @ v_aug_j over j<=i in PSUM;
    VectorE takes 1/denominator and applies it during the final
    evacuation; one batched DMA per 512-row chunk stores fp16 output
    (host upcasts to f32)

Loop-timing structure: the For_i body is unrolled 4x — each For_i
iteration carries an all-engine barrier in its reset block (~6us of
drain + pipeline/DMA-prefetch refill), so fewer, fatter iterations
amortize it. qT/kT/vT/v and x tiles are multi-buffered so consecutive
bodies overlap (next body's DMA/casts/projections run under the current
body's scores/AV tail).
"""

import math
import os

import numpy as np

B = 8
S = 2048
E = 2048
D = 128
P = 128
NE = E // P  # 16 contraction chunks
NS = S // P  # 16 sequence blocks
ST = 512  # s-tile width for projections / score chunks
NST = S // ST  # 4
VW = D + 1  # logical v block width incl. ones column
VSTRIDE = D + 1  # physical stride of v blocks in SBUF
SCALE = 1.0 / math.sqrt(S)
NEG = -1.0e30
NEG16 = -60000.0  # fp16-representable; exp underflows to exactly 0

_PROGRAMS = {}

# which phases to emit (for microbenchmarking): subset of
# {"proj", "vtrans", "scores", "av", "store"}
PHASES = frozenset(
    p
    for p in os.environ.get("K_PHASES", "proj,vtrans,scores,av,store").split(",")
    if p
)

# per-projection matmul precision; see module docstring
PROJ_DTYPE = os.environ.get("K_PROJ_DTYPE", "mixed")
W_SCALE = 256.0  # host pre-scale of W before fp8 quantization (2**8: exact)
# produce the fp8 x copy on-device (Pool engine) instead of shipping it
CAST_ON_DEVICE = os.environ.get("K_CAST", "1") == "1"
# comma-separated rotation of engines hosting the cast ops
CAST_ENG = os.environ.get("K_CAST_ENG", "vector")

# tunables: PSUM bank split (proj, sc, out; vt fixed at 1) and probs pool depth
PS_CFG = tuple(int(v) for v in os.environ.get("K_PS_CFG", "3,2,2").split(","))
# v natural-block transpose: "pe" (tensor-engine transpose + DVE copy) or
# "dma" (SBUF->SBUF DMA transpose, frees PE/DVE/PSUM)
VTRANS = os.environ.get("K_VTRANS", "pe")
VTRANS_DMA = os.environ.get("K_VTRANS_DMA", "sync")
PPOOL = int(os.environ.get("K_PPOOL", "24"))
OUT_DMA = os.environ.get("K_OUT_DMA", "gpsimd")  # engine hosting output stores
OUT_DTYPE = os.environ.get("K_OUT_DTYPE", "fp16")  # f32|fp16 store dtype
X_DMA = os.environ.get("K_X_DMA", "sync")  # engine hosting x loads
XV_DMA = os.environ.get("K_XV_DMA", "")  # separate queue for the fp16 x stream
QKV_BUFS = int(os.environ.get("K_QKV_BUFS", "2"))
X_BUFS = int(os.environ.get("K_X_BUFS", "5"))
# causal mask of the diagonal score block: "pe" accumulates an
# identity-matmul of the fp16 mask into the score PSUM (keeps the exp
# critical path off the DVE); "dve" adds the f32 mask on the VectorE
MASK_ON_PE = os.environ.get("K_MASK", "pe")
# engine applying 1/denominator during the AV evacuation: the ScalarE is
# idle during the AV phase (exp done), so hosting the multiply there
# turns out_ps banks around without queueing behind DVE work
NORM_ENG = os.environ.get("K_NORM", "vector")


def _dtypes(mybir):
    """(x dtype for q/k proj, x dtype for v proj, matmul/probs dtype)."""
    f16 = mybir.dt.float16
    f8 = mybir.dt.float8e4
    if PROJ_DTYPE == "fp8":
        return f8, f8, mybir.dt.bfloat16
    if PROJ_DTYPE == "mixed":
        return f8, f16, f16
    if PROJ_DTYPE == "fp16":
        return f16, f16, f16
    return mybir.dt.bfloat16, mybir.dt.bfloat16, mybir.dt.bfloat16


def build_program(iters=1):
    global _PROGRAMS
    key = (
        iters, PHASES, PROJ_DTYPE, PS_CFG, PPOOL, OUT_DMA, OUT_DTYPE,
        X_DMA, XV_DMA, QKV_BUFS, X_BUFS, CAST_ON_DEVICE, CAST_ENG,
        VTRANS, VTRANS_DMA, MASK_ON_PE, NORM_ENG,
    )
    if key in _PROGRAMS:
        return _PROGRAMS[key]

    import concourse.bacc as bacc
    import concourse.mybir as mybir
    import concourse.tile as tile
    from concourse.masks import make_identity

    f32 = mybir.dt.float32

    nc = bacc.Bacc("TRN2", target_bir_lowering=False, debug=False)

    qk_dt, v_dt, mmdt = _dtypes(mybir)
    split_x = qk_dt != v_dt
    ship_x8 = split_x and not CAST_ON_DEVICE
    odt = mybir.dt.float16 if OUT_DTYPE == "fp16" else f32

    # q/k/v weights and biases arrive host-packed into single tensors in
    # the exact SBUF tile layout
    if split_x:
        xv_d = nc.dram_tensor("xT16", [E, S], v_dt, kind="ExternalInput")
        xq_d = (
            nc.dram_tensor("xT8", [E, S], qk_dt, kind="ExternalInput")
            if ship_x8
            else None
        )
        wqk_d = nc.dram_tensor("wqk", [P, 2 * NE * D], qk_dt, kind="ExternalInput")
        wv_d = nc.dram_tensor("wv", [P, NE * D], v_dt, kind="ExternalInput")
    else:
        xq_d = xv_d = nc.dram_tensor("xT", [E, S], qk_dt, kind="ExternalInput")
        wqk_d = nc.dram_tensor("wqkv", [P, 3 * NE * D], qk_dt, kind="ExternalInput")
        wv_d = None
    bias_d = nc.dram_tensor("bias", [D, 3], f32, kind="ExternalInput")
    out_d = nc.dram_tensor("out", [S, D], odt, kind="ExternalOutput")

    with tile.TileContext(nc) as tc:
        with (
            tc.tile_pool(name="const", bufs=1) as cpool,
            tc.tile_pool(name="xq", bufs=X_BUFS) as xqpool,
            tc.tile_pool(name="xv", bufs=X_BUFS) as xvpool,
            tc.tile_pool(name="qkv", bufs=QKV_BUFS) as qkvpool,
            tc.tile_pool(name="probs", bufs=PPOOL) as ppool,
            tc.tile_pool(name="osb", bufs=int(os.environ.get("K_OSB_BUFS", "2"))) as opool,
            tc.tile_pool(name="misc", bufs=2) as mpool,
            tc.tile_pool(name="proj_ps", bufs=PS_CFG[0], space="PSUM") as proj_ps,
            tc.tile_pool(name="sc_ps", bufs=PS_CFG[1], space="PSUM") as sc_ps,
            tc.tile_pool(name="vt_ps", bufs=1, space="PSUM") as vt_ps,
            tc.tile_pool(name="out_ps", bufs=PS_CFG[2], space="PSUM") as out_ps,
        ):
            # ---- iteration-invariant setup ----
            ident = cpool.tile([P, P], mmdt, tag="ident")
            make_identity(nc, ident[:])
            # cmaskT[k_local, q_local]: 0 where q >= k (valid), -1e30 where q < k
            cm_dt = f32 if MASK_ON_PE == "dve" else mmdt
            cm_fill = NEG if MASK_ON_PE == "dve" else NEG16
            cmaskT = cpool.tile([P, P], cm_dt, tag="cmaskT")
            nc.gpsimd.memset(cmaskT[:], 0.0)
            nc.gpsimd.affine_select(
                out=cmaskT[:],
                in_=cmaskT[:],
                compare_op=mybir.AluOpType.is_ge,
                fill=cm_fill,
                base=0,
                # iota[r, c] = c - r ; keep (0.0) where c - r >= 0
                pattern=[[1, P]],
                channel_multiplier=-1,
            )

            w_sb = {}
            if split_x:
                wqk_sb = cpool.tile([P, 2 * NE * D], qk_dt, tag="wqk")
                nc.sync.dma_start(wqk_sb[:], wqk_d[:, :])
                wv_sb = cpool.tile([P, NE * D], v_dt, tag="wv")
                nc.sync.dma_start(wv_sb[:], wv_d[:, :])
                w_sb["q"] = wqk_sb[:, 0 : NE * D]
                w_sb["k"] = wqk_sb[:, NE * D : 2 * NE * D]
                w_sb["v"] = wv_sb[:]
            else:
                wqkv_sb = cpool.tile([P, 3 * NE * D], qk_dt, tag="wqkv")
                nc.sync.dma_start(wqkv_sb[:], wqk_d[:, :])
                for n, pj in enumerate(("q", "k", "v")):
                    w_sb[pj] = wqkv_sb[:, n * NE * D : (n + 1) * NE * D]
            bias_sb = cpool.tile([P, 3], f32, tag="bias")
            nc.sync.dma_start(bias_sb[:], bias_d[:, :])
            b_sb = {pj: bias_sb[:, n : n + 1] for n, pj in enumerate(("q", "k", "v"))}

            # HAM warmup while the first DMAs land (runs once, cold)
            wps = proj_ps.tile([P, ST], f32, name="warm", tag="proj")
            for wi in range(40):
                nc.tensor.matmul(
                    wps[:, 0:P],
                    lhsT=ident[:],
                    rhs=ident[:],
                    start=(wi == 0),
                    stop=(wi == 39),
                )

            def body():
                _emit_body(
                    nc,
                    mybir,
                    pools={
                        "xqpool": xqpool,
                        "xvpool": xvpool,
                        "qkvpool": qkvpool,
                        "ppool": ppool,
                        "opool": opool,
                        "mpool": mpool,
                        "proj_ps": proj_ps,
                        "sc_ps": sc_ps,
                        "vt_ps": vt_ps,
                        "out_ps": out_ps,
                    },
                    dram={"xq": xq_d, "xv": xv_d, "out": out_d},
                    consts={
                        "ident": ident,
                        "cmaskT": cmaskT,
                        "w": w_sb,
                        "b": b_sb,
                    },
                )

            # unrolling the loop body halves the per-iteration all-engine
            # barrier count (the For_i reset barrier blocks cross-iteration
            # DMA prefetch and drains the PE pipeline)
            unroll = int(os.environ.get("K_UNROLL", "4"))
            if iters > 1:
                n_loop = iters // unroll
                rem = iters - n_loop * unroll
                hints = (
                    mybir.EngineType.PE,
                    mybir.EngineType.Activation,
                    mybir.EngineType.DVE,
                    mybir.EngineType.SP,
                    mybir.EngineType.Pool,
                )
                if n_loop > 1:
                    with tc.For_i(0, n_loop, 1, hint_engines=hints):
                        for _ in range(unroll):
                            body()
                else:
                    rem = iters
                for _ in range(rem):
                    body()
            else:
                body()

    nc.compile()
    _PROGRAMS[key] = nc
    return nc


def _emit_body(nc, mybir, pools, dram, consts):
    f32 = mybir.dt.float32
    qk_dt, v_dt, mmdt = _dtypes(mybir)
    split_x = qk_dt != v_dt
    ship_x8 = split_x and not CAST_ON_DEVICE
    fp8 = mybir.dt.float8e4

    xqpool = pools["xqpool"]
    xvpool = pools["xvpool"]
    qkvpool = pools["qkvpool"]
    ppool = pools["ppool"]
    opool = pools["opool"]
    mpool = pools["mpool"]
    proj_ps = pools["proj_ps"]
    sc_ps = pools["sc_ps"]
    vt_ps = pools["vt_ps"]
    out_ps = pools["out_ps"]
    xq_d = dram["xq"]
    xv_d = dram["xv"]
    out_d = dram["out"]
    ident = consts["ident"]
    cmaskT = consts["cmaskT"]
    w_sb = consts["w"]
    b_sb = consts["b"]

    x_eng = getattr(nc, X_DMA)
    xv_eng = getattr(nc, XV_DMA) if XV_DMA else x_eng
    cast_engs = [getattr(nc, e) for e in CAST_ENG.split(",")]

    # ---- xT loads: one tile per s-tile, prefetchable across iterations ----
    xv_v = xv_d.rearrange("(ec p) s -> p ec s", p=P)
    xq_v = xq_d.rearrange("(ec p) s -> p ec s", p=P) if ship_x8 else None
    xq_st = []
    xv_st = []
    for st in range(NST):
        xt16 = xvpool.tile([P, NE * ST], v_dt, name=f"xv{st}", tag="xv")
        if st == 0 and os.environ.get("K_FINE", "1") == "1":
            # fine-grained pieces: the first projection chain can start
            # after ~1 piece instead of waiting for the full tile
            for ec in range(NE):
                xv_eng.dma_start(
                    xt16[:, ec * ST : (ec + 1) * ST],
                    xv_v[:, ec, st * ST : (st + 1) * ST],
                )
        else:
            xv_eng.dma_start(
                xt16[:].rearrange("p (ec s) -> p ec s", ec=NE),
                xv_v[:, :, st * ST : (st + 1) * ST],
            )
        xv_st.append(xt16)
        if not split_x:
            xq_st.append(xt16)
        elif ship_x8:
            xt8 = xqpool.tile([P, NE * ST], qk_dt, name=f"xq{st}", tag="xq")
            if st == 0:
                for ec in range(NE):
                    x_eng.dma_start(
                        xt8[:, ec * ST : (ec + 1) * ST],
                        xq_v[:, ec, st * ST : (st + 1) * ST],
                    )
            else:
                x_eng.dma_start(
                    xt8[:].rearrange("p (ec s) -> p ec s", ec=NE),
                    xq_v[:, :, st * ST : (st + 1) * ST],
                )
            xq_st.append(xt8)
        else:
            # device-side fp16 -> fp8 cast: tiles allocated here, the cast
            # ops themselves are emitted just-in-time inside the s-tile
            # loop so they don't serialize ahead of the PSUM evacuations
            # in the in-order engine queues
            xt8 = xqpool.tile([P, NE * ST], fp8, name=f"xq{st}", tag="xq")
            xq_st.append(xt8)

    def emit_casts(st):
        """fp16 -> fp8 casts for tile st, pairs rotated across engines."""
        if not (split_x and not ship_x8):
            return
        for g in range(NE // 2):
            eng = cast_engs[g % len(cast_engs)]
            dst = xq_st[st][:, 2 * g * ST : (2 * g + 2) * ST]
            src = xv_st[st][:, 2 * g * ST : (2 * g + 2) * ST]
            if hasattr(eng, "tensor_copy"):
                eng.tensor_copy(dst, src)
            else:
                eng.copy(dst, src)

    qT_sb = qkvpool.tile([P, S], mmdt, tag="qT")
    kT_sb = qkvpool.tile([P, S], mmdt, tag="kT")
    vT_sb = qkvpool.tile([P, S], mmdt, tag="vT")
    v_sb = qkvpool.tile([P, NS * VSTRIDE], mmdt, tag="v")
    dest = {"q": qT_sb, "k": kT_sb, "v": vT_sb}

    # ones column of v_aug
    for sb in range(NS):
        nc.vector.memset(v_sb[:, sb * VSTRIDE + D : sb * VSTRIDE + D + 1], 1.0)

    def emit_proj(pj, st):
        """Project chunk st of x onto head dim for q/k/v; write dest[pj]."""
        x_sb = xq_st[st] if pj in ("q", "k") else xv_st[st]
        xdt = qk_dt if pj in ("q", "k") else v_dt
        ps = proj_ps.tile([P, ST], f32, tag="proj")
        if xdt == fp8:
            for g in range(NE // 2):
                nc.tensor.matmul(
                    ps[:],
                    lhsT=w_sb[pj][:, 2 * g * D : (2 * g + 2) * D].rearrange(
                        "p (i d) -> p i d", i=2
                    ),
                    rhs=x_sb[:, 2 * g * ST : (2 * g + 2) * ST].rearrange(
                        "p (i s) -> p i s", i=2
                    ),
                    start=(g == 0),
                    stop=(g == NE // 2 - 1),
                    perf_mode=mybir.MatmulPerfMode.DoubleRow,
                )
            nc.vector.tensor_scalar(
                dest[pj][:, st * ST : (st + 1) * ST],
                ps[:],
                1.0 / W_SCALE,
                b_sb[pj],
                op0=mybir.AluOpType.mult,
                op1=mybir.AluOpType.add,
            )
        else:
            for ec in range(NE):
                nc.tensor.matmul(
                    ps[:],
                    lhsT=w_sb[pj][:, ec * D : (ec + 1) * D],
                    rhs=x_sb[:, ec * ST : (ec + 1) * ST],
                    start=(ec == 0),
                    stop=(ec == NE - 1),
                )
            nc.vector.tensor_scalar_add(
                dest[pj][:, st * ST : (st + 1) * ST],
                ps[:],
                b_sb[pj],
            )

    probs_pieces = {}

    def emit_piece(c, j):
        qs = max(c * ST, j * P)
        w = (c + 1) * ST - qs
        sps = sc_ps.tile([P, ST], f32, tag="sc")
        diag = j * P >= c * ST
        nc.tensor.matmul(
            sps[:, :w],
            lhsT=kT_sb[:, j * P : (j + 1) * P],
            rhs=qT_sb[:, qs : qs + w],
            start=True,
            stop=not (diag and MASK_ON_PE == "pe"),
            skip_group_check=True,
        )
        if diag:
            if MASK_ON_PE == "pe":
                nc.tensor.matmul(
                    sps[:, 0:P],
                    lhsT=ident[:],
                    rhs=cmaskT[:],
                    start=False,
                    stop=True,
                    skip_group_check=True,
                )
            else:
                nc.vector.tensor_add(sps[:, 0:P], sps[:, 0:P], cmaskT[:])
        prb = ppool.tile([P, ST], mmdt, name="prb", tag="probs")
        nc.scalar.activation(
            prb[:, :w],
            sps[:, :w],
            func=mybir.ActivationFunctionType.Exp,
            bias=0.0,
            scale=SCALE,
        )
        probs_pieces[(j, c)] = (prb, qs)

    odt = mybir.dt.float16 if OUT_DTYPE == "fp16" else f32
    out_v = out_d.rearrange("(c i p) d -> p c i d", p=P, i=ST // P)
    for st in range(NST):
        # casts for tile 0 must precede its projections; casts for later
        # tiles are emitted one tile ahead (end of the previous section)
        if st == 0:
            emit_casts(0)

        # ---- projections for this s-tile ----
        # order q, k first so the score/exp stream for this chunk can
        # start while the v projection and transposes still run
        for pj in ("q", "k") if "proj" in PHASES else ():
            emit_proj(pj, st)

        # ---- scoresT + exp for q-chunk c = st, interleaved with the v
        # projection: the score stream is paced by the ScalarE exp
        # evacuations (~3x slower than the PE matmul per piece), so the
        # independent v-projection chunks keep the PE busy in between ----
        c = st
        npieces = 4 * c + 4 if "scores" in PHASES else 0
        x_sb = xv_st[st]
        vps = None
        if "proj" in PHASES:
            vps = proj_ps.tile([P, ST], f32, name="vps", tag="proj")
        nv = NE if "proj" in PHASES else 0
        vdone = 0
        for j in range(npieces):
            emit_piece(c, j)
            vtarget = (j + 1) * nv // npieces
            while vdone < vtarget:
                ec = vdone
                nc.tensor.matmul(
                    vps[:],
                    lhsT=w_sb["v"][:, ec * D : (ec + 1) * D],
                    rhs=x_sb[:, ec * ST : (ec + 1) * ST],
                    start=(ec == 0),
                    stop=(ec == NE - 1),
                    skip_group_check=True,
                )
                vdone += 1
        if nv:
            while vdone < nv:
                ec = vdone
                nc.tensor.matmul(
                    vps[:],
                    lhsT=w_sb["v"][:, ec * D : (ec + 1) * D],
                    rhs=x_sb[:, ec * ST : (ec + 1) * ST],
                    start=(ec == 0),
                    stop=(ec == NE - 1),
                    skip_group_check=True,
                )
                vdone += 1
            nc.vector.tensor_scalar_add(
                vT_sb[:, st * ST : (st + 1) * ST],
                vps[:],
                b_sb["v"],
            )

        # prefetch-cast the next tile while this tile's scores/AV run
        if st + 1 < NST:
            emit_casts(st + 1)

        # ---- v natural blocks (PE transpose of vT, or DMA transpose) ----
        for sb in (
            range(st * (ST // P), (st + 1) * (ST // P)) if "vtrans" in PHASES else ()
        ):
            if VTRANS == "dma":
                getattr(nc, VTRANS_DMA).dma_start_transpose(
                    v_sb[:, sb * VSTRIDE : sb * VSTRIDE + D],
                    vT_sb[:, sb * P : (sb + 1) * P],
                )
            else:
                tp = vt_ps.tile([P, P], mmdt, tag="vt")
                nc.tensor.transpose(tp[:], vT_sb[:, sb * P : (sb + 1) * P], ident[:])
                nc.vector.tensor_copy(v_sb[:, sb * VSTRIDE : sb * VSTRIDE + D], tp[:])

        # ---- AV + normalize + store for the 4 q-blocks of chunk c ----
        osb = opool.tile([P, (ST // P) * D], odt, tag="osb")
        for i in range(4 * c, 4 * c + 4) if "av" in PHASES else ():
            ops = out_ps.tile([P, D + 1], f32, tag="out")
            for j in range(i + 1):
                prb, qs = probs_pieces[(j, c)]
                off = i * P - qs
                nc.tensor.matmul(
                    ops[:],
                    lhsT=prb[:, off : off + P],
                    rhs=v_sb[:, j * VSTRIDE : j * VSTRIDE + VW],
                    start=(j == 0),
                    stop=(j == i),
                )
            recip = mpool.tile([P, 1], f32, tag="recip")
            nc.vector.reciprocal(recip[:], ops[:, D : D + 1])
            il = i - 4 * c
            if NORM_ENG == "scalar":
                nc.scalar.activation(
                    osb[:, il * D : (il + 1) * D],
                    ops[:, 0:D],
                    func=mybir.ActivationFunctionType.Copy,
                    bias=0.0,
                    scale=recip[:, 0:1],
                )
            else:
                nc.vector.tensor_scalar_mul(
                    osb[:, il * D : (il + 1) * D], ops[:, 0:D], recip[:, 0:1]
                )
        # one batched store for the whole 512-row chunk
        if "av" in PHASES:
            eng = getattr(nc, OUT_DMA)
            eng.dma_start(
                out_v[:, c, :, :],
                osb[:].rearrange("p (i d) -> p i d", i=ST // P),
            )


def make_in_maps(x, Wq, bq, Wk, bk, Wv, bv):
    import ml_dtypes

    fp8 = ml_dtypes.float8_e4m3
    f16 = np.float16
    bf16 = ml_dtypes.bfloat16
    x = np.asarray(x, dtype=np.float32)

    def wcast(W, dt, scale):
        wt = np.asarray(W, dtype=np.float32).T * scale  # [E, D]
        packed = wt.reshape(NE, P, D).transpose(1, 0, 2).reshape(P, NE * D)
        return np.ascontiguousarray(packed).astype(dt)

    bias = np.ascontiguousarray(
        np.stack([np.asarray(b, dtype=np.float32) for b in (bq, bk, bv)], axis=1)
    )
    if PROJ_DTYPE == "mixed":
        shared = {
            "wqk": np.ascontiguousarray(
                np.concatenate(
                    [wcast(Wq, fp8, W_SCALE), wcast(Wk, fp8, W_SCALE)], axis=1
                )
            ),
            "wv": wcast(Wv, f16, 1.0),
            "bias": bias,
        }
        maps = []
        for b in range(B):
            m = {"xT16": np.ascontiguousarray(x[b].T).astype(f16), **shared}
            if not CAST_ON_DEVICE:
                m["xT8"] = np.ascontiguousarray(x[b].T).astype(fp8)
            maps.append(m)
        return maps

    if PROJ_DTYPE == "fp8":
        xdt, wscale = fp8, W_SCALE
    elif PROJ_DTYPE == "fp16":
        xdt, wscale = f16, 1.0
    else:
        xdt, wscale = bf16, 1.0
    shared = {
        "wqkv": np.ascontiguousarray(
            np.concatenate(
                [wcast(W, xdt, wscale) for W in (Wq, Wk, Wv)], axis=1
            )
        ),
        "bias": bias,
    }
    return [
        {"xT": np.ascontiguousarray(x[b].T).astype(xdt), **shared}
        for b in range(B)
    ]


def kernel(x, Wq, bq, Wk, bk, Wv, bv):
    from concourse.bass_utils import run_bass_kernel_spmd

    nc = build_program()
    in_maps = make_in_maps(x, Wq, bq, Wk, bk, Wv, bv)
    res = run_bass_kernel_spmd(nc, in_maps, list(range(B)))
    return np.stack(
        [res.results[i]["out"].astype(np.float32) for i in range(B)], axis=0
    )
